# revision 11
# baseline (speedup 1.0000x reference)
"""PINN loss kernel for trn2 (8 NeuronCores, data-parallel over points).

v2: forward-mode AD with a single Laplacian second-derivative stream
(s = s_xx + s_yy; the losses only need u_xx+u_yy / v_xx+v_yy), bf16
streams + matmuls, layer-1 tangent scales folded into pre-scaled weight
copies (host-side), engine-balanced elementwise work.

Layout: features on partitions, points on the free dim; per-core shard of
6250 points processed in 13 tiles of <=512 points. Per-core partial sums
[128, 8] are combined on host (the unshard step).
"""

import os
import sys

for _p in ("/opt/trn_rl_repo", "/root/.axon_site/_ro/trn_rl_repo"):
    if os.path.isdir(_p) and _p not in sys.path:
        sys.path.insert(0, _p)

import numpy as np
import ml_dtypes
from contextlib import ExitStack

from concourse import bass, bacc, tile, mybir
from concourse.bass_utils import run_bass_kernel_spmd

NCORES = 8
NPTS = 50000
NPC = NPTS // NCORES          # 6250 points per core
FD = 512                      # points per tile
NT = (NPC + FD - 1) // FD     # 13 tiles (12 full + one of 106)
NCH = (NPC + 127) // 128      # 49 transpose chunks per core
NS = 14                       # output slots per point
NU = 1.56e-05
EPS = 1e-08
SQRT2 = 1.4142135623730951

F32 = mybir.dt.float32
BF = mybir.dt.float16
AF = mybir.ActivationFunctionType
ALU = mybir.AluOpType
BF_NP = np.float16

DBG_NT = int(os.environ.get("PINN_NT", NT))          # tiles to emit (debug)
DBG_L4 = os.environ.get("PINN_SKIP_L4", "") == ""    # emit L4+transpose
DBG_PW = os.environ.get("PINN_SKIP_PW", "") == ""    # emit pointwise phase

_CACHE = {}


def _build():
    nc = bacc.Bacc("TRN2", target_bir_lowering=False, debug=False)

    # ---- DRAM I/O ----
    d_feat = nc.dram_tensor("featb", [8, NPC], BF, kind="ExternalInput")
    d_w0 = nc.dram_tensor("w0b", [8, 128], BF, kind="ExternalInput")
    d_wh = nc.dram_tensor("wh", [128, 128 * 3], BF, kind="ExternalInput")
    d_wx = nc.dram_tensor("wx", [128, 128 * 3], BF, kind="ExternalInput")
    d_w4 = nc.dram_tensor("w4sp", [128, 4 * NS], BF, kind="ExternalInput")
    d_bias = nc.dram_tensor("bias", [128, 4], F32, kind="ExternalInput")
    d_b4s = nc.dram_tensor("b4s", [NS, 1], F32, kind="ExternalInput")
    d_cnb = nc.dram_tensor("cnb", [128, 8], F32, kind="ExternalInput")
    d_tins = nc.dram_tensor("tins", [128, 6 * NCH], F32, kind="ExternalInput")
    d_mask = nc.dram_tensor("mask", [128, NCH], F32, kind="ExternalInput")
    d_id = nc.dram_tensor("ident", [NS, NS], F32, kind="ExternalInput")
    d_out = nc.dram_tensor("sout", [128, 8], F32, kind="ExternalOutput")

    with tile.TileContext(nc) as tc, ExitStack() as ctx:
        wp = ctx.enter_context(tc.tile_pool(name="wp", bufs=1))
        sb = ctx.enter_context(tc.tile_pool(name="sb", bufs=int(os.environ.get("PINN_SBUFS", "4"))))
        scr = ctx.enter_context(tc.tile_pool(name="scr", bufs=26))
        pa = ctx.enter_context(tc.tile_pool(name="pa", bufs=2, space="PSUM"))
        pb = ctx.enter_context(tc.tile_pool(name="pb", bufs=2, space="PSUM"))

        # ---- persistent sbuf tensors ----
        feat = wp.tile([8, NPC], BF, tag="feat")
        w0 = wp.tile([8, 128], BF, tag="w0")
        wh = wp.tile([128, 128 * 3], BF, tag="wh")
        wx = wp.tile([128, 128 * 3], BF, tag="wx")
        w4 = wp.tile([128, 4 * NS], BF, tag="w4")
        bias = wp.tile([128, 4], F32, tag="bias")
        b4s = wp.tile([NS, 1], F32, tag="b4s")
        cnb = wp.tile([128, 8], F32, tag="cnb")
        tins = wp.tile([128, 6 * NCH], F32, tag="tins")
        mask = wp.tile([128, NCH], F32, tag="mask")
        ident = wp.tile([NS, NS], F32, tag="ident")
        qall = wp.tile([128, NS * NCH], F32, tag="qall")
        souts = wp.tile([128, 8], F32, tag="souts")

        warm = wp.tile([1, 8], F32, tag="warm")
        nc.gpsimd.memset(warm[:, :], 0.25)
        nc.scalar.activation(warm[:, :], warm[:, :], AF.Tanh)
        nc.scalar.activation(warm[:, :], warm[:, :], AF.Square)
        dma = nc.sync.dma_start
        dma(out=feat[:, :], in_=d_feat[:, :])
        dma(out=w0[:, :], in_=d_w0[:, :])
        dma(out=wh[:, :], in_=d_wh[:, :])
        dma(out=wx[:, :], in_=d_wx[:, :])
        dma(out=w4[:, :], in_=d_w4[:, :])
        dma(out=bias[:, :], in_=d_bias[:, :])
        dma(out=b4s[:, :], in_=d_b4s[:, :])
        dma(out=cnb[:, :], in_=d_cnb[:, :])
        dma(out=tins[:, :], in_=d_tins[:, :])
        dma(out=mask[:, :], in_=d_mask[:, :])
        dma(out=ident[:, :], in_=d_id[:, :])
        nc.gpsimd.memset(qall[:, :], 0.0)

        V, S, G, T = nc.vector, nc.scalar, nc.gpsimd, nc.tensor
        SGN = [-1.0, 1.0, -1.0]  # sign of stored pp term per hidden iter

        def emit_el1(t):
            off = t * FD
            fd = min(FD, NPC - off)

            # ---- L0 matmul + EL1; outputs packed into comb0 [128, 4fd] ----
            zh = pa.tile([128, fd], F32, tag="pa", name=f"zh0_{t}")
            T.matmul(zh[:, :], w0[:, :], feat[:, off:off + fd],
                     start=True, stop=True)
            comb = sb.tile([128, 4 * fd], BF, tag="comb", name=f"comb0_{t}")
            th = comb[:, 0:fd]
            qm1 = comb[:, fd:2 * fd]
            thgm = comb[:, 3 * fd:4 * fd]
            S.activation(th, zh[:, :], AF.Tanh, bias=bias[:, 0:1])
            q = sb.tile([128, fd], BF, tag="q", name=f"q0_{t}")
            S.activation(q[:, :], th, AF.Square)
            V.tensor_scalar_add(qm1, q[:, :], -1.0)
            G.tensor_tensor(thgm, qm1, th, ALU.mult)
            return {"comb": comb, "pend": None}

        # ---- hidden iteration li (weights W1,W2,W3) ----
        def emit_hidden(t, li, st):
            off = t * FD
            fd = min(FD, NPC - off)
            flush_pend(st)
            pc = st["comb"]
            W = wh[:, li * 128:(li + 1) * 128]
            zh = pa.tile([128, fd], F32, tag="pa", name=f"zh{li}_{t}")
            zz = pb.tile([128, 3 * fd], F32, tag="pb", name=f"zz{li}_{t}")
            if li == 0:
                T.matmul(zh[:, :], W, pc[:, 0:fd], start=True, stop=True)
                T.matmul(zz[:, 0:fd], wx[:, 0:128], pc[:, fd:2 * fd],
                         start=True, stop=True)
                T.matmul(zz[:, fd:2 * fd], wx[:, 128:256], pc[:, fd:2 * fd],
                         start=True, stop=True)
                T.matmul(zz[:, 2 * fd:3 * fd], wx[:, 256:384],
                         pc[:, 3 * fd:4 * fd], start=True, stop=True)
            else:
                T.matmul(zh[:, :], W, pc[:, 0:fd], start=True, stop=True)
                T.matmul(zz[:, 0:fd], W, pc[:, fd:2 * fd],
                         start=True, stop=True)
                T.matmul(zz[:, fd:2 * fd], W, pc[:, 2 * fd:3 * fd],
                         start=True, stop=True)
                T.matmul(zz[:, 2 * fd:3 * fd], W, pc[:, 3 * fd:4 * fd],
                         start=True, stop=True)

            comb = sb.tile([128, 4 * fd], BF, tag="comb", name=f"comb{li + 1}_{t}")
            th = comb[:, 0:fd]
            S.activation(th, zh[:, :], AF.Tanh, bias=bias[:, li + 1:li + 2])
            q = sb.tile([128, fd], BF, tag="q", name=f"q{li + 1}_{t}")
            S.activation(q[:, :], th, AF.Square)
            qm1 = sb.tile([128, fd], BF, tag="qm1", name=f"qm1{li}_{t}")
            V.tensor_scalar_add(qm1[:, :], q[:, :], -1.0)
            qb = q[:, :].unsqueeze(1).broadcast_to([128, 3, fd])
            V.scalar_tensor_tensor(
                comb[:, fd:4 * fd].rearrange("p (r f) -> p r f", r=3), qb, 1.0,
                zz[:, :].rearrange("p (r f) -> p r f", r=3),
                ALU.subtract, ALU.mult)
            zx2 = sb.tile([128, 2 * fd], BF, tag="zx2", name=f"zx2{li}_{t}")
            S.activation(zx2[:, :], zz[:, 0:2 * fd], AF.Square, scale=SQRT2)
            zx2s = sb.tile([128, fd], BF, tag="zx2s", name=f"zx2s{li}_{t}")
            G.tensor_tensor(zx2s[:, :], zx2[:, 0:fd], zx2[:, fd:2 * fd],
                            ALU.add)
            thgm = sb.tile([128, fd], BF, tag="thgm", name=f"thgm{li}_{t}")
            G.tensor_tensor(thgm[:, :], qm1[:, :], th, ALU.mult)
            st["comb"] = comb
            st["pend"] = (t, li, thgm, zx2s, comb)

        def flush_pend(st):
            # pp of the previous block + signed merge into comb's s slot
            if st["pend"] is None:
                return
            t, li, thgm, zx2s, comb = st["pend"]
            st["pend"] = None
            fd = min(FD, NPC - t * FD)
            pp = sb.tile([128, fd], BF, tag="pp", name=f"pp{li}_{t}")
            V.tensor_tensor(pp[:, :], thgm[:, :], zx2s[:, :], ALU.mult)
            op = ALU.add if li == 1 else ALU.subtract
            eng = G if li == 1 else V
            eng.tensor_tensor(comb[:, 3 * fd:4 * fd], comb[:, 3 * fd:4 * fd],
                              pp[:, :], op)

        def emit_tail(t, st):
            if not DBG_L4:
                return
            off = t * FD
            fd = min(FD, NPC - off)
            nchunks = (fd + 127) // 128
            flush_pend(st)
            comb = st["comb"]

            # ---- L4: stacked output matmuls into o14 [NS, fd] ----
            o14 = pa.tile([NS, fd], F32, tag="pa", name=f"o14_{t}")
            T.matmul(o14[:, :], w4[:, 0:NS], comb[:, 0:fd],
                     start=True, stop=False)
            T.matmul(o14[:, :], w4[:, NS:2 * NS], comb[:, fd:2 * fd],
                     start=False, stop=False)
            T.matmul(o14[:, :], w4[:, 2 * NS:3 * NS], comb[:, 2 * fd:3 * fd],
                     start=False, stop=False)
            T.matmul(o14[:, :], w4[:, 3 * NS:4 * NS], comb[:, 3 * fd:4 * fd],
                     start=False, stop=True)

            o14sb = sb.tile([NS, fd], F32, tag="o14sb", name=f"o14sb_{t}")
            S.activation(o14sb[:, :], o14[:, :], AF.Identity, bias=b4s[:, 0:1])

            # ---- transpose to points-on-partitions ----
            qt = pa.tile([128, NS * nchunks], F32, tag="pa", name=f"qt_{t}")
            for ci in range(nchunks):
                w = min(128, fd - ci * 128)
                T.transpose(qt[0:w, ci * NS:(ci + 1) * NS],
                            o14sb[:, ci * 128:ci * 128 + w],
                            ident[:, :])
            gw = min(128, fd - (nchunks - 1) * 128)
            qoff = t * NS * 4
            if gw == 128:
                S.copy(qall[:, qoff:qoff + NS * nchunks], qt[:, :])
            else:
                if nchunks > 1:
                    S.copy(qall[:, qoff:qoff + NS * (nchunks - 1)],
                           qt[:, 0:NS * (nchunks - 1)])
                S.copy(
                    qall[0:gw, qoff + NS * (nchunks - 1):qoff + NS * nchunks],
                    qt[0:gw, NS * (nchunks - 1):NS * nchunks])

        for p in range(0, DBG_NT, 2):
            ts = [t for t in (p, p + 1) if t < DBG_NT]
            sts = {}
            for t in ts:
                sts[t] = emit_el1(t)
            for li in range(3):
                for t in ts:
                    emit_hidden(t, li, sts[t])
            for t in ts:
                emit_tail(t, sts[t])

        # ---- pointwise loss phase on [128, NCH] views ----
        def _pointwise():
            def qv(j):
                return qall[:, j:NS * NCH:NS]

            _ctr = [0]

            def new():
                _ctr[0] += 1
                return scr.tile([128, NCH], F32, tag="scr", name=f"scr{_ctr[0]}")

            def tt(a, b, op, eng=V):
                o = new()
                eng.tensor_tensor(o[:, :], a, b, op)
                return o

            A = tt(qv(4), qv(8), ALU.add, G)             # ux+uy
            B = tt(qv(5), qv(9), ALU.add, V)             # vx+vy
            uv1 = tt(qv(1), A[:, :], ALU.mult, G)        # v*(ux+uy)
            uv2 = tt(qv(0), B[:, :], ALU.mult, V)        # u*(vx+vy)
            uvxy = tt(uv1[:, :], uv2[:, :], ALU.add, V)

            def stt_nu(zz, eng):
                o = new()
                eng.scalar_tensor_tensor(o[:, :], qv(3), NU, zz, ALU.add,
                                         ALU.mult)
                return o

            t1 = stt_nu(qv(12), V)               # (nut+NU)*Lu
            t3 = tt(qv(7), qv(4), ALU.mult, V)   # nux*ux
            t4 = tt(qv(11), qv(8), ALU.mult, G)  # nuy*uy
            a1 = tt(uvxy[:, :], qv(6), ALU.add, V)
            a3 = tt(t3[:, :], t4[:, :], ALU.add, G)
            a4 = tt(t1[:, :], a3[:, :], ALU.add, V)
            f_u = tt(a1[:, :], a4[:, :], ALU.subtract, V)

            t5 = stt_nu(qv(13), V)               # (nut+NU)*Lv
            t7 = tt(qv(7), qv(5), ALU.mult, V)   # nux*vx
            t8 = tt(qv(11), qv(9), ALU.mult, G)  # nuy*vy
            b1 = tt(uvxy[:, :], qv(10), ALU.add, V)
            b3 = tt(t7[:, :], t8[:, :], ALU.add, G)
            b4 = tt(t5[:, :], b3[:, :], ALU.add, V)
            f_v = tt(b1[:, :], b4[:, :], ALU.subtract, V)

            ic = tt(qv(4), qv(9), ALU.add, G)

            t1b = new()
            V.tensor_scalar(t1b[:, :], qv(0), cnb[:, 0:1], cnb[:, 1:2],
                            ALU.mult, ALU.add)
            xnb = new()
            V.tensor_scalar(xnb[:, :], tins[:, 0:NCH], cnb[:, 2:3], cnb[:, 3:4],
                            ALU.mult, ALU.add)
            t2b = new()
            V.tensor_scalar(t2b[:, :], qv(1), cnb[:, 4:5], cnb[:, 5:6],
                            ALU.mult, ALU.add)
            ynb = new()
            V.tensor_scalar(ynb[:, :], tins[:, NCH:2 * NCH], cnb[:, 6:7],
                            cnb[:, 7:8], ALU.mult, ALU.add)
            m1 = tt(t1b[:, :], xnb[:, :], ALU.mult, V)
            m2 = tt(t2b[:, :], ynb[:, :], ALU.mult, G)
            bc0 = tt(m1[:, :], m2[:, :], ALU.add, V)
            bc = tt(bc0[:, :], mask[:, :], ALU.mult, V)

            du = tt(tins[:, 2 * NCH:3 * NCH], qv(0), ALU.subtract, V)
            dv = tt(tins[:, 3 * NCH:4 * NCH], qv(1), ALU.subtract, G)
            dp = tt(tins[:, 4 * NCH:5 * NCH], qv(2), ALU.subtract, V)
            dnut = tt(tins[:, 5 * NCH:6 * NCH], qv(3), ALU.subtract, G)

            for k, val in enumerate([f_u, f_v, bc, ic, du, dv, dp, dnut]):
                o = new()
                S.activation(o[:, :], val[:, :], AF.Square,
                             accum_out=souts[:, k:k + 1])

            nc.sync.dma_start(out=d_out[:, :], in_=souts[:, :])

        if DBG_PW:
            _pointwise()
        else:
            nc.sync.dma_start(out=d_out[:, :], in_=qall[:, 0:8])

    nc.compile()
    return nc


def _prep_core(inputs, c):
    s = slice(c * NPC, (c + 1) * NPC)
    f32 = np.float32
    col = lambda k: np.asarray(inputs[k], f32)[s, 0]
    feat = np.ascontiguousarray(np.stack([
        col("x"), col("y"), col("x_normal"), col("y_normal"), col("sdf"),
        col("gamma_1"), col("gamma_2"), col("gamma_3")]))
    W = [np.asarray(inputs[f"W{i}"], f32) for i in range(5)]
    b = [np.asarray(inputs[f"b{i}"], f32) for i in range(5)]
    cn = np.asarray(inputs["coef_norm"], f32)

    w0x, w0y = W[0][0, :], W[0][1, :]
    w0s = 2.0 * (w0x * w0x + w0y * w0y)
    wh = np.concatenate([W[1], W[2], W[3]], axis=1)
    wx = np.concatenate([W[1] * w0x[:, None], W[1] * w0y[:, None],
                         W[1] * w0s[:, None]], axis=1)
    w4sp = np.zeros((128, 4 * NS), f32)
    w4sp[:, 0:4] = W[4]                       # slot th -> rows 0:4
    w4sp[:, NS + 4:NS + 8] = W[4]             # slot t_x -> rows 4:8
    w4sp[:, 2 * NS + 8:2 * NS + 12] = W[4]    # slot t_y -> rows 8:12
    w4sp[:, 3 * NS + 12:3 * NS + 14] = -W[4][:, 0:2]  # slot s_in -> rows 12:14

    bias = np.stack([b[0], b[1], b[2], b[3]], axis=1)
    b4s = np.concatenate([b[4], np.zeros(NS - 4, f32)])[:, None]
    cnv = np.array([cn[3, 0] + EPS, cn[2, 0], cn[1, 5] + EPS, cn[0, 5],
                    cn[3, 1] + EPS, cn[2, 1], cn[1, 6] + EPS, cn[0, 6]], f32)
    cnb = np.broadcast_to(cnv, (128, 8)).copy()

    def tcol(k):
        a = np.zeros(NCH * 128, f32)
        a[:NPC] = col(k)
        return a.reshape(NCH, 128).T  # [128, NCH]

    tins = np.ascontiguousarray(np.concatenate(
        [tcol("x_normal"), tcol("y_normal"), tcol("u0"), tcol("v0"),
         tcol("p0"), tcol("nut0")], axis=1))
    m = np.zeros(NCH * 128, f32)
    m[:NPC] = 1.0
    mask = np.ascontiguousarray(m.reshape(NCH, 128).T)

    bf = lambda a: np.ascontiguousarray(a.astype(BF_NP))
    return {
        "featb": bf(feat), "w0b": bf(W[0]), "wh": bf(wh), "wx": bf(wx),
        "w4sp": bf(w4sp),
        "bias": np.ascontiguousarray(bias), "b4s": b4s,
        "cnb": cnb, "tins": tins, "mask": mask,
        "ident": np.eye(NS, dtype=f32),
    }


def _get_nc():
    if "nc" not in _CACHE:
        _CACHE["nc"] = _build()
    return _CACHE["nc"]


def run_device(inputs, **kw):
    nc = _get_nc()
    in_maps = [_prep_core(inputs, c) for c in range(NCORES)]
    res = run_bass_kernel_spmd(nc, in_maps, core_ids=list(range(NCORES)), **kw)
    return res


def _combine(results):
    S = np.stack([r["sout"] for r in results]).astype(np.float64)  # [8,128,8]
    m = S.sum(axis=(0, 1)) / NPTS
    rans, bcl, icl = m[0] + m[1], m[2], m[3]
    ul, vl, pl, nl = m[4], m[5], m[6], m[7]
    inlet = ul + vl + pl + nl
    total = rans + bcl + inlet + icl
    return np.array([total, rans, bcl, inlet, icl, ul, vl, pl, nl],
                    dtype=np.float32)


def kernel(**inputs):
    res = run_device(inputs)
    return _combine(res.results)


# revision 12
# speedup vs baseline: 1.0930x; 1.0930x over previous
"""PINN loss kernel for trn2 (8 NeuronCores, data-parallel over points).

v2: forward-mode AD with a single Laplacian second-derivative stream
(s = s_xx + s_yy; the losses only need u_xx+u_yy / v_xx+v_yy), bf16
streams + matmuls, layer-1 tangent scales folded into pre-scaled weight
copies (host-side), engine-balanced elementwise work.

Layout: features on partitions, points on the free dim; per-core shard of
6250 points processed in 13 tiles of <=512 points. Per-core partial sums
[128, 8] are combined on host (the unshard step).
"""

import os
import sys

for _p in ("/opt/trn_rl_repo", "/root/.axon_site/_ro/trn_rl_repo"):
    if os.path.isdir(_p) and _p not in sys.path:
        sys.path.insert(0, _p)

import numpy as np
import ml_dtypes
from contextlib import ExitStack

from concourse import bass, bacc, tile, mybir
from concourse.bass_utils import run_bass_kernel_spmd

NCORES = 8
NPTS = 50000
NPC = NPTS // NCORES          # 6250 points per core
FD = 512                      # points per tile
NT = (NPC + FD - 1) // FD     # 13 tiles (12 full + one of 106)
NCH = (NPC + 127) // 128      # 49 transpose chunks per core
NS = 14                       # output slots per point
NU = 1.56e-05
EPS = 1e-08
SQRT2 = 1.4142135623730951

F32 = mybir.dt.float32
BF = mybir.dt.float16
AF = mybir.ActivationFunctionType
ALU = mybir.AluOpType
BF_NP = np.float16

DBG_NT = int(os.environ.get("PINN_NT", NT))          # tiles to emit (debug)
DBG_L4 = os.environ.get("PINN_SKIP_L4", "") == ""    # emit L4+transpose
DBG_PW = os.environ.get("PINN_SKIP_PW", "") == ""    # emit pointwise phase

_CACHE = {}


def _build():
    nc = bacc.Bacc("TRN2", target_bir_lowering=False, debug=False)

    # ---- DRAM I/O ----
    d_feat = nc.dram_tensor("featb", [8, NPC], BF, kind="ExternalInput")
    d_w0 = nc.dram_tensor("w0b", [8, 128], BF, kind="ExternalInput")
    d_wh = nc.dram_tensor("wh", [128, 128 * 3], BF, kind="ExternalInput")
    d_wx = nc.dram_tensor("wx", [128, 128 * 3], BF, kind="ExternalInput")
    d_w2n = nc.dram_tensor("w2n", [128, 128], BF, kind="ExternalInput")
    d_w4 = nc.dram_tensor("w4sp", [128, 5 * NS], BF, kind="ExternalInput")
    d_bias = nc.dram_tensor("bias", [128, 4], F32, kind="ExternalInput")
    d_b4s = nc.dram_tensor("b4s", [NS, 1], F32, kind="ExternalInput")
    d_cnb = nc.dram_tensor("cnb", [128, 8], F32, kind="ExternalInput")
    d_tins = nc.dram_tensor("tins", [128, 6 * NCH], F32, kind="ExternalInput")
    d_mask = nc.dram_tensor("mask", [128, NCH], F32, kind="ExternalInput")
    d_id = nc.dram_tensor("ident", [NS, NS], F32, kind="ExternalInput")
    d_out = nc.dram_tensor("sout", [128, 8], F32, kind="ExternalOutput")

    with tile.TileContext(nc) as tc, ExitStack() as ctx:
        wp = ctx.enter_context(tc.tile_pool(name="wp", bufs=1))
        sb = ctx.enter_context(tc.tile_pool(name="sb", bufs=int(os.environ.get("PINN_SBUFS", "4"))))
        scr = ctx.enter_context(tc.tile_pool(name="scr", bufs=26))
        pa = ctx.enter_context(tc.tile_pool(name="pa", bufs=2, space="PSUM"))
        pb = ctx.enter_context(tc.tile_pool(name="pb", bufs=2, space="PSUM"))

        # ---- persistent sbuf tensors ----
        feat = wp.tile([8, NPC], BF, tag="feat")
        w0 = wp.tile([8, 128], BF, tag="w0")
        wh = wp.tile([128, 128 * 3], BF, tag="wh")
        wx = wp.tile([128, 128 * 3], BF, tag="wx")
        w2n = wp.tile([128, 128], BF, tag="w2n")
        w4 = wp.tile([128, 5 * NS], BF, tag="w4")
        bias = wp.tile([128, 4], F32, tag="bias")
        b4s = wp.tile([NS, 1], F32, tag="b4s")
        cnb = wp.tile([128, 8], F32, tag="cnb")
        tins = wp.tile([128, 6 * NCH], F32, tag="tins")
        mask = wp.tile([128, NCH], F32, tag="mask")
        ident = wp.tile([NS, NS], F32, tag="ident")
        qall = wp.tile([128, NS * NCH], F32, tag="qall")
        souts = wp.tile([128, 8], F32, tag="souts")

        warm = wp.tile([1, 8], F32, tag="warm")
        nc.gpsimd.memset(warm[:, :], 0.25)
        nc.scalar.activation(warm[:, :], warm[:, :], AF.Tanh)
        nc.scalar.activation(warm[:, :], warm[:, :], AF.Square)
        dma = nc.sync.dma_start
        dma(out=feat[:, :], in_=d_feat[:, :])
        dma(out=w0[:, :], in_=d_w0[:, :])
        dma(out=wh[:, :], in_=d_wh[:, :])
        dma(out=wx[:, :], in_=d_wx[:, :])
        dma(out=w2n[:, :], in_=d_w2n[:, :])
        dma(out=w4[:, :], in_=d_w4[:, :])
        dma(out=bias[:, :], in_=d_bias[:, :])
        dma(out=b4s[:, :], in_=d_b4s[:, :])
        dma(out=cnb[:, :], in_=d_cnb[:, :])
        dma(out=tins[:, :], in_=d_tins[:, :])
        dma(out=mask[:, :], in_=d_mask[:, :])
        dma(out=ident[:, :], in_=d_id[:, :])
        nc.gpsimd.memset(qall[:, :], 0.0)

        V, S, G, T = nc.vector, nc.scalar, nc.gpsimd, nc.tensor
        SGN = [-1.0, 1.0, -1.0]  # sign of stored pp term per hidden iter

        def emit_el1(t):
            off = t * FD
            fd = min(FD, NPC - off)

            # ---- L0 matmul + EL1; outputs packed into comb0 [128, 4fd] ----
            zh = pa.tile([128, fd], F32, tag="pa", name=f"zh0_{t}")
            T.matmul(zh[:, :], w0[:, :], feat[:, off:off + fd],
                     start=True, stop=True)
            comb = sb.tile([128, 4 * fd], BF, tag="comb", name=f"comb0_{t}")
            th = comb[:, 0:fd]
            qm1 = comb[:, fd:2 * fd]
            thgm = comb[:, 3 * fd:4 * fd]
            S.activation(th, zh[:, :], AF.Tanh, bias=bias[:, 0:1])
            q = sb.tile([128, fd], BF, tag="q", name=f"q0_{t}")
            S.activation(q[:, :], th, AF.Square)
            V.tensor_scalar_add(qm1, q[:, :], -1.0)
            G.tensor_tensor(thgm, qm1, th, ALU.mult)
            return {"comb": comb, "pp": None}

        # ---- hidden iteration li (weights W1,W2,W3) ----
        def emit_hidden(t, li, st):
            off = t * FD
            fd = min(FD, NPC - off)
            pc = st["comb"]
            W = wh[:, li * 128:(li + 1) * 128]
            zh = pa.tile([128, fd], F32, tag="pa", name=f"zh{li}_{t}")
            zz = pb.tile([128, 3 * fd], F32, tag="pb", name=f"zz{li}_{t}")
            if li == 0:
                T.matmul(zh[:, :], W, pc[:, 0:fd], start=True, stop=True)
                T.matmul(zz[:, 0:fd], wx[:, 0:128], pc[:, fd:2 * fd],
                         start=True, stop=True)
                T.matmul(zz[:, fd:2 * fd], wx[:, 128:256], pc[:, fd:2 * fd],
                         start=True, stop=True)
                T.matmul(zz[:, 2 * fd:3 * fd], wx[:, 256:384],
                         pc[:, 3 * fd:4 * fd], start=True, stop=True)
            else:
                T.matmul(zh[:, :], W, pc[:, 0:fd], start=True, stop=True)
                T.matmul(zz[:, 0:fd], W, pc[:, fd:2 * fd],
                         start=True, stop=True)
                T.matmul(zz[:, fd:2 * fd], W, pc[:, 2 * fd:3 * fd],
                         start=True, stop=True)
                T.matmul(zz[:, 2 * fd:3 * fd], W, pc[:, 3 * fd:4 * fd],
                         start=True, stop=False)
                Wpp = w2n[:, :] if li == 1 else W
                T.matmul(zz[:, 2 * fd:3 * fd], Wpp, st["pp"][:, :],
                         start=False, stop=True)

            comb = sb.tile([128, 4 * fd], BF, tag="comb", name=f"comb{li + 1}_{t}")
            th = comb[:, 0:fd]
            S.activation(th, zh[:, :], AF.Tanh, bias=bias[:, li + 1:li + 2])
            q = sb.tile([128, fd], BF, tag="q", name=f"q{li + 1}_{t}")
            S.activation(q[:, :], th, AF.Square)
            qm1 = sb.tile([128, fd], BF, tag="qm1", name=f"qm1{li}_{t}")
            V.tensor_scalar_add(qm1[:, :], q[:, :], -1.0)
            qb = q[:, :].unsqueeze(1).broadcast_to([128, 3, fd])
            V.scalar_tensor_tensor(
                comb[:, fd:4 * fd].rearrange("p (r f) -> p r f", r=3), qb, 1.0,
                zz[:, :].rearrange("p (r f) -> p r f", r=3),
                ALU.subtract, ALU.mult)
            zx2 = sb.tile([128, 2 * fd], BF, tag="zx2", name=f"zx2{li}_{t}")
            S.activation(zx2[:, :], zz[:, 0:2 * fd], AF.Square, scale=SQRT2)
            zx2s = sb.tile([128, fd], BF, tag="zx2s", name=f"zx2s{li}_{t}")
            G.tensor_tensor(zx2s[:, :], zx2[:, 0:fd], zx2[:, fd:2 * fd],
                            ALU.add)
            thgm = sb.tile([128, fd], BF, tag="thgm", name=f"thgm{li}_{t}")
            G.tensor_tensor(thgm[:, :], qm1[:, :], th, ALU.mult)
            pp = sb.tile([128, fd], BF, tag="pp", name=f"pp{li}_{t}")
            V.tensor_tensor(pp[:, :], thgm[:, :], zx2s[:, :], ALU.mult)
            st["comb"] = comb
            st["pp"] = pp

        def emit_tail(t, st):
            if not DBG_L4:
                return
            off = t * FD
            fd = min(FD, NPC - off)
            nchunks = (fd + 127) // 128
            comb = st["comb"]

            # ---- L4: stacked output matmuls into o14 [NS, fd] ----
            o14 = pa.tile([NS, fd], F32, tag="pa", name=f"o14_{t}")
            T.matmul(o14[:, :], w4[:, 0:NS], comb[:, 0:fd],
                     start=True, stop=False)
            T.matmul(o14[:, :], w4[:, NS:2 * NS], comb[:, fd:2 * fd],
                     start=False, stop=False)
            T.matmul(o14[:, :], w4[:, 2 * NS:3 * NS], comb[:, 2 * fd:3 * fd],
                     start=False, stop=False)
            T.matmul(o14[:, :], w4[:, 3 * NS:4 * NS], comb[:, 3 * fd:4 * fd],
                     start=False, stop=False)
            T.matmul(o14[:, :], w4[:, 4 * NS:5 * NS], st["pp"][:, :],
                     start=False, stop=True)

            o14sb = sb.tile([NS, fd], F32, tag="o14sb", name=f"o14sb_{t}")
            S.activation(o14sb[:, :], o14[:, :], AF.Identity, bias=b4s[:, 0:1])

            # ---- transpose to points-on-partitions ----
            qt = pa.tile([128, NS * nchunks], F32, tag="pa", name=f"qt_{t}")
            for ci in range(nchunks):
                w = min(128, fd - ci * 128)
                T.transpose(qt[0:w, ci * NS:(ci + 1) * NS],
                            o14sb[:, ci * 128:ci * 128 + w],
                            ident[:, :])
            gw = min(128, fd - (nchunks - 1) * 128)
            qoff = t * NS * 4
            if gw == 128:
                S.copy(qall[:, qoff:qoff + NS * nchunks], qt[:, :])
            else:
                if nchunks > 1:
                    S.copy(qall[:, qoff:qoff + NS * (nchunks - 1)],
                           qt[:, 0:NS * (nchunks - 1)])
                S.copy(
                    qall[0:gw, qoff + NS * (nchunks - 1):qoff + NS * nchunks],
                    qt[0:gw, NS * (nchunks - 1):NS * nchunks])

        sts = {}
        prev_ts = []
        for p in range(0, DBG_NT, 2):
            ts = [t for t in (p, p + 1) if t < DBG_NT]
            for t in ts:
                sts[t] = emit_el1(t)
            for t in ts:
                emit_hidden(t, 0, sts[t])
            for t in prev_ts:
                emit_tail(t, sts[t])
            for t in ts:
                emit_hidden(t, 1, sts[t])
            for t in ts:
                emit_hidden(t, 2, sts[t])
            prev_ts = ts
        for t in prev_ts:
            emit_tail(t, sts[t])

        # ---- pointwise loss phase on [128, NCH] views ----
        def _pointwise():
            def qv(j):
                return qall[:, j:NS * NCH:NS]

            _ctr = [0]

            def new():
                _ctr[0] += 1
                return scr.tile([128, NCH], F32, tag="scr", name=f"scr{_ctr[0]}")

            def tt(a, b, op, eng=V):
                o = new()
                eng.tensor_tensor(o[:, :], a, b, op)
                return o

            A = tt(qv(4), qv(8), ALU.add, G)             # ux+uy
            B = tt(qv(5), qv(9), ALU.add, V)             # vx+vy
            uv1 = tt(qv(1), A[:, :], ALU.mult, G)        # v*(ux+uy)
            uv2 = tt(qv(0), B[:, :], ALU.mult, V)        # u*(vx+vy)
            uvxy = tt(uv1[:, :], uv2[:, :], ALU.add, V)

            def stt_nu(zz, eng):
                o = new()
                eng.scalar_tensor_tensor(o[:, :], qv(3), NU, zz, ALU.add,
                                         ALU.mult)
                return o

            t1 = stt_nu(qv(12), V)               # (nut+NU)*Lu
            t3 = tt(qv(7), qv(4), ALU.mult, V)   # nux*ux
            t4 = tt(qv(11), qv(8), ALU.mult, G)  # nuy*uy
            a1 = tt(uvxy[:, :], qv(6), ALU.add, V)
            a3 = tt(t3[:, :], t4[:, :], ALU.add, G)
            a4 = tt(t1[:, :], a3[:, :], ALU.add, V)
            f_u = tt(a1[:, :], a4[:, :], ALU.subtract, V)

            t5 = stt_nu(qv(13), V)               # (nut+NU)*Lv
            t7 = tt(qv(7), qv(5), ALU.mult, V)   # nux*vx
            t8 = tt(qv(11), qv(9), ALU.mult, G)  # nuy*vy
            b1 = tt(uvxy[:, :], qv(10), ALU.add, V)
            b3 = tt(t7[:, :], t8[:, :], ALU.add, G)
            b4 = tt(t5[:, :], b3[:, :], ALU.add, V)
            f_v = tt(b1[:, :], b4[:, :], ALU.subtract, V)

            ic = tt(qv(4), qv(9), ALU.add, G)

            t1b = new()
            V.tensor_scalar(t1b[:, :], qv(0), cnb[:, 0:1], cnb[:, 1:2],
                            ALU.mult, ALU.add)
            xnb = new()
            V.tensor_scalar(xnb[:, :], tins[:, 0:NCH], cnb[:, 2:3], cnb[:, 3:4],
                            ALU.mult, ALU.add)
            t2b = new()
            V.tensor_scalar(t2b[:, :], qv(1), cnb[:, 4:5], cnb[:, 5:6],
                            ALU.mult, ALU.add)
            ynb = new()
            V.tensor_scalar(ynb[:, :], tins[:, NCH:2 * NCH], cnb[:, 6:7],
                            cnb[:, 7:8], ALU.mult, ALU.add)
            m1 = tt(t1b[:, :], xnb[:, :], ALU.mult, V)
            m2 = tt(t2b[:, :], ynb[:, :], ALU.mult, G)
            bc0 = tt(m1[:, :], m2[:, :], ALU.add, V)
            bc = tt(bc0[:, :], mask[:, :], ALU.mult, V)

            du = tt(tins[:, 2 * NCH:3 * NCH], qv(0), ALU.subtract, V)
            dv = tt(tins[:, 3 * NCH:4 * NCH], qv(1), ALU.subtract, G)
            dp = tt(tins[:, 4 * NCH:5 * NCH], qv(2), ALU.subtract, V)
            dnut = tt(tins[:, 5 * NCH:6 * NCH], qv(3), ALU.subtract, G)

            for k, val in enumerate([f_u, f_v, bc, ic, du, dv, dp, dnut]):
                o = new()
                S.activation(o[:, :], val[:, :], AF.Square,
                             accum_out=souts[:, k:k + 1])

            nc.sync.dma_start(out=d_out[:, :], in_=souts[:, :])

        if DBG_PW:
            _pointwise()
        else:
            nc.sync.dma_start(out=d_out[:, :], in_=qall[:, 0:8])

    nc.compile()
    return nc


def _prep_core(inputs, c):
    s = slice(c * NPC, (c + 1) * NPC)
    f32 = np.float32
    col = lambda k: np.asarray(inputs[k], f32)[s, 0]
    feat = np.ascontiguousarray(np.stack([
        col("x"), col("y"), col("x_normal"), col("y_normal"), col("sdf"),
        col("gamma_1"), col("gamma_2"), col("gamma_3")]))
    W = [np.asarray(inputs[f"W{i}"], f32) for i in range(5)]
    b = [np.asarray(inputs[f"b{i}"], f32) for i in range(5)]
    cn = np.asarray(inputs["coef_norm"], f32)

    w0x, w0y = W[0][0, :], W[0][1, :]
    w0s = 2.0 * (w0x * w0x + w0y * w0y)
    wh = np.concatenate([W[1], W[2], W[3]], axis=1)
    wx = np.concatenate([W[1] * w0x[:, None], W[1] * w0y[:, None],
                         W[1] * w0s[:, None]], axis=1)
    w4sp = np.zeros((128, 5 * NS), f32)
    w4sp[:, 0:4] = W[4]                       # slot th -> rows 0:4
    w4sp[:, NS + 4:NS + 8] = W[4]             # slot t_x -> rows 4:8
    w4sp[:, 2 * NS + 8:2 * NS + 12] = W[4]    # slot t_y -> rows 8:12
    w4sp[:, 3 * NS + 12:3 * NS + 14] = -W[4][:, 0:2]  # slot s -> rows 12:14
    w4sp[:, 4 * NS + 12:4 * NS + 14] = W[4][:, 0:2]   # slot pp -> rows 12:14

    bias = np.stack([b[0], b[1], b[2], b[3]], axis=1)
    b4s = np.concatenate([b[4], np.zeros(NS - 4, f32)])[:, None]
    cnv = np.array([cn[3, 0] + EPS, cn[2, 0], cn[1, 5] + EPS, cn[0, 5],
                    cn[3, 1] + EPS, cn[2, 1], cn[1, 6] + EPS, cn[0, 6]], f32)
    cnb = np.broadcast_to(cnv, (128, 8)).copy()

    def tcol(k):
        a = np.zeros(NCH * 128, f32)
        a[:NPC] = col(k)
        return a.reshape(NCH, 128).T  # [128, NCH]

    tins = np.ascontiguousarray(np.concatenate(
        [tcol("x_normal"), tcol("y_normal"), tcol("u0"), tcol("v0"),
         tcol("p0"), tcol("nut0")], axis=1))
    m = np.zeros(NCH * 128, f32)
    m[:NPC] = 1.0
    mask = np.ascontiguousarray(m.reshape(NCH, 128).T)

    bf = lambda a: np.ascontiguousarray(a.astype(BF_NP))
    return {
        "featb": bf(feat), "w0b": bf(W[0]), "wh": bf(wh), "wx": bf(wx),
        "w2n": bf(-W[2]),
        "w4sp": bf(w4sp),
        "bias": np.ascontiguousarray(bias), "b4s": b4s,
        "cnb": cnb, "tins": tins, "mask": mask,
        "ident": np.eye(NS, dtype=f32),
    }


def _get_nc():
    if "nc" not in _CACHE:
        _CACHE["nc"] = _build()
    return _CACHE["nc"]


def run_device(inputs, **kw):
    nc = _get_nc()
    in_maps = [_prep_core(inputs, c) for c in range(NCORES)]
    res = run_bass_kernel_spmd(nc, in_maps, core_ids=list(range(NCORES)), **kw)
    return res


def _combine(results):
    S = np.stack([r["sout"] for r in results]).astype(np.float64)  # [8,128,8]
    m = S.sum(axis=(0, 1)) / NPTS
    rans, bcl, icl = m[0] + m[1], m[2], m[3]
    ul, vl, pl, nl = m[4], m[5], m[6], m[7]
    inlet = ul + vl + pl + nl
    total = rans + bcl + inlet + icl
    return np.array([total, rans, bcl, inlet, icl, ul, vl, pl, nl],
                    dtype=np.float32)


def kernel(**inputs):
    res = run_device(inputs)
    return _combine(res.results)


# revision 13
# speedup vs baseline: 1.3293x; 1.2162x over previous
"""PINN loss kernel for trn2 (8 NeuronCores, data-parallel over points).

v2: forward-mode AD with a single Laplacian second-derivative stream
(s = s_xx + s_yy; the losses only need u_xx+u_yy / v_xx+v_yy), bf16
streams + matmuls, layer-1 tangent scales folded into pre-scaled weight
copies (host-side), engine-balanced elementwise work.

Layout: features on partitions, points on the free dim; per-core shard of
6250 points processed in 13 tiles of <=512 points. Per-core partial sums
[128, 8] are combined on host (the unshard step).
"""

import os
import sys

for _p in ("/opt/trn_rl_repo", "/root/.axon_site/_ro/trn_rl_repo"):
    if os.path.isdir(_p) and _p not in sys.path:
        sys.path.insert(0, _p)

import numpy as np
import ml_dtypes
from contextlib import ExitStack

from concourse import bass, bacc, tile, mybir
from concourse.bass_utils import run_bass_kernel_spmd

NCORES = 8
NPTS = 50000
NPC = NPTS // NCORES          # 6250 points per core
FD = 512                      # points per tile
NT = (NPC + FD - 1) // FD     # 13 tiles (12 full + one of 106)
NCH = (NPC + 127) // 128      # 49 transpose chunks per core
NS = 14                       # output slots per point
NU = 1.56e-05
EPS = 1e-08
SQRT2 = 1.4142135623730951

F32 = mybir.dt.float32
BF = mybir.dt.float16
AF = mybir.ActivationFunctionType
ALU = mybir.AluOpType
BF_NP = np.float16

DBG_NT = int(os.environ.get("PINN_NT", NT))          # tiles to emit (debug)
DBG_L4 = os.environ.get("PINN_SKIP_L4", "") == ""    # emit L4+transpose
DBG_PW = os.environ.get("PINN_SKIP_PW", "") == ""    # emit pointwise phase

_CACHE = {}


def _build():
    nc = bacc.Bacc("TRN2", target_bir_lowering=False, debug=False)

    # ---- DRAM I/O ----
    d_feat = nc.dram_tensor("featb", [8, NPC], BF, kind="ExternalInput")
    d_w0 = nc.dram_tensor("w0b", [8, 128], BF, kind="ExternalInput")
    d_wh = nc.dram_tensor("wh", [128, 128 * 3], BF, kind="ExternalInput")
    d_wx = nc.dram_tensor("wx", [128, 128 * 3], BF, kind="ExternalInput")
    d_w2n = nc.dram_tensor("w2n", [128, 128], BF, kind="ExternalInput")
    d_w4 = nc.dram_tensor("w4sp", [128, 5 * NS], BF, kind="ExternalInput")
    d_bias = nc.dram_tensor("bias", [128, 4], F32, kind="ExternalInput")
    d_b4s = nc.dram_tensor("b4s", [NS, 1], F32, kind="ExternalInput")
    d_cnb = nc.dram_tensor("cnb", [128, 8], F32, kind="ExternalInput")
    d_tins = nc.dram_tensor("tins", [128, 6 * NCH], F32, kind="ExternalInput")
    d_mask = nc.dram_tensor("mask", [128, NCH], F32, kind="ExternalInput")
    d_id = nc.dram_tensor("ident", [NS, NS], F32, kind="ExternalInput")
    d_out = nc.dram_tensor("sout", [128, 8], F32, kind="ExternalOutput")

    with tile.TileContext(nc) as tc, ExitStack() as ctx:
        wp = ctx.enter_context(tc.tile_pool(name="wp", bufs=1))
        sb = ctx.enter_context(tc.tile_pool(name="sb", bufs=int(os.environ.get("PINN_SBUFS", "6"))))
        scr = ctx.enter_context(tc.tile_pool(name="scr", bufs=26))
        pa = ctx.enter_context(tc.tile_pool(name="pa", bufs=2, space="PSUM"))
        pb = ctx.enter_context(tc.tile_pool(name="pb", bufs=2, space="PSUM"))

        # ---- persistent sbuf tensors ----
        feat = wp.tile([8, NPC], BF, tag="feat")
        w0 = wp.tile([8, 128], BF, tag="w0")
        wh = wp.tile([128, 128 * 3], BF, tag="wh")
        wx = wp.tile([128, 128 * 3], BF, tag="wx")
        w2n = wp.tile([128, 128], BF, tag="w2n")
        w4 = wp.tile([128, 5 * NS], BF, tag="w4")
        bias = wp.tile([128, 4], F32, tag="bias")
        b4s = wp.tile([NS, 1], F32, tag="b4s")
        cnb = wp.tile([128, 8], F32, tag="cnb")
        tins = wp.tile([128, 6 * NCH], F32, tag="tins")
        mask = wp.tile([128, NCH], F32, tag="mask")
        ident = wp.tile([NS, NS], F32, tag="ident")
        qall = wp.tile([128, NS * NCH], F32, tag="qall")
        souts = wp.tile([128, 8], F32, tag="souts")

        warm = wp.tile([1, 8], F32, tag="warm")
        nc.gpsimd.memset(warm[:, :], 0.25)
        nc.scalar.activation(warm[:, :], warm[:, :], AF.Tanh)
        nc.scalar.activation(warm[:, :], warm[:, :], AF.Square)
        dma = nc.sync.dma_start
        dma(out=feat[:, :], in_=d_feat[:, :])
        dma(out=w0[:, :], in_=d_w0[:, :])
        dma(out=wh[:, :], in_=d_wh[:, :])
        dma(out=wx[:, :], in_=d_wx[:, :])
        dma(out=w2n[:, :], in_=d_w2n[:, :])
        dma(out=w4[:, :], in_=d_w4[:, :])
        dma(out=bias[:, :], in_=d_bias[:, :])
        dma(out=b4s[:, :], in_=d_b4s[:, :])
        dma(out=cnb[:, :], in_=d_cnb[:, :])
        dma(out=tins[:, :], in_=d_tins[:, :])
        dma(out=mask[:, :], in_=d_mask[:, :])
        dma(out=ident[:, :], in_=d_id[:, :])
        nc.gpsimd.memset(qall[:, :], 0.0)

        V, S, G, T = nc.vector, nc.scalar, nc.gpsimd, nc.tensor
        SGN = [-1.0, 1.0, -1.0]  # sign of stored pp term per hidden iter

        def emit_el1(t):
            off = t * FD
            fd = min(FD, NPC - off)

            # ---- L0 matmul + EL1; outputs packed into comb0 [128, 4fd] ----
            zh = pa.tile([128, fd], F32, tag="pa", name=f"zh0_{t}")
            T.matmul(zh[:, :], w0[:, :], feat[:, off:off + fd],
                     start=True, stop=True)
            comb = sb.tile([128, 4 * fd], BF, tag="comb", name=f"comb0_{t}")
            th = comb[:, 0:fd]
            qm1 = comb[:, fd:2 * fd]
            thgm = comb[:, 3 * fd:4 * fd]
            S.activation(th, zh[:, :], AF.Tanh, bias=bias[:, 0:1])
            q = sb.tile([128, fd], BF, tag="q", name=f"q0_{t}")
            S.activation(q[:, :], th, AF.Square)
            V.tensor_scalar_add(qm1, q[:, :], -1.0)
            G.tensor_tensor(thgm, qm1, th, ALU.mult)
            return {"comb": comb, "pp": None}

        # ---- hidden iteration li (weights W1,W2,W3) ----
        def emit_hidden(t, li, st):
            off = t * FD
            fd = min(FD, NPC - off)
            pc = st["comb"]
            W = wh[:, li * 128:(li + 1) * 128]
            zh = pa.tile([128, fd], F32, tag="pa", name=f"zh{li}_{t}")
            zz = pb.tile([128, 3 * fd], F32, tag="pb", name=f"zz{li}_{t}")
            if li == 0:
                T.matmul(zh[:, :], W, pc[:, 0:fd], start=True, stop=True)
                T.matmul(zz[:, 0:fd], wx[:, 0:128], pc[:, fd:2 * fd],
                         start=True, stop=True)
                T.matmul(zz[:, fd:2 * fd], wx[:, 128:256], pc[:, fd:2 * fd],
                         start=True, stop=True)
                T.matmul(zz[:, 2 * fd:3 * fd], wx[:, 256:384],
                         pc[:, 3 * fd:4 * fd], start=True, stop=True)
            else:
                T.matmul(zh[:, :], W, pc[:, 0:fd], start=True, stop=True)
                T.matmul(zz[:, 0:fd], W, pc[:, fd:2 * fd],
                         start=True, stop=True)
                T.matmul(zz[:, fd:2 * fd], W, pc[:, 2 * fd:3 * fd],
                         start=True, stop=True)
                T.matmul(zz[:, 2 * fd:3 * fd], W, pc[:, 3 * fd:4 * fd],
                         start=True, stop=False)
                Wpp = w2n[:, :] if li == 1 else W
                T.matmul(zz[:, 2 * fd:3 * fd], Wpp, st["pp"][:, :],
                         start=False, stop=True)

            comb = sb.tile([128, 4 * fd], BF, tag="comb", name=f"comb{li + 1}_{t}")
            th = comb[:, 0:fd]
            S.activation(th, zh[:, :], AF.Tanh, bias=bias[:, li + 1:li + 2])
            q = sb.tile([128, fd], BF, tag="q", name=f"q{li + 1}_{t}")
            S.activation(q[:, :], th, AF.Square)
            qm1 = sb.tile([128, fd], BF, tag="qm1", name=f"qm1{li}_{t}")
            V.tensor_scalar_add(qm1[:, :], q[:, :], -1.0)
            qb = q[:, :].unsqueeze(1).broadcast_to([128, 3, fd])
            V.scalar_tensor_tensor(
                comb[:, fd:4 * fd].rearrange("p (r f) -> p r f", r=3), qb, 1.0,
                zz[:, :].rearrange("p (r f) -> p r f", r=3),
                ALU.subtract, ALU.mult)
            zx2 = sb.tile([128, 2 * fd], BF, tag="zx2", name=f"zx2{li}_{t}")
            S.activation(zx2[:, :], zz[:, 0:2 * fd], AF.Square, scale=SQRT2)
            thgm = sb.tile([128, fd], BF, tag="thgm", name=f"thgm{li}_{t}")
            G.tensor_tensor(thgm[:, :], qm1[:, :], th, ALU.mult)
            zx2s = sb.tile([128, fd], BF, tag="zx2s", name=f"zx2s{li}_{t}")
            V.tensor_tensor(zx2s[:, :], zx2[:, 0:fd], zx2[:, fd:2 * fd],
                            ALU.add)
            pp = sb.tile([128, fd], BF, tag="pp", name=f"pp{li}_{t}")
            G.tensor_tensor(pp[:, :], thgm[:, :], zx2s[:, :], ALU.mult)
            st["comb"] = comb
            st["pp"] = pp

        def emit_tail(t, st):
            if not DBG_L4:
                return
            off = t * FD
            fd = min(FD, NPC - off)
            nchunks = (fd + 127) // 128
            comb = st["comb"]

            # ---- L4: stacked output matmuls into o14 [NS, fd] ----
            o14 = pa.tile([NS, fd], F32, tag="pa", name=f"o14_{t}")
            T.matmul(o14[:, :], w4[:, 0:NS], comb[:, 0:fd],
                     start=True, stop=False)
            T.matmul(o14[:, :], w4[:, NS:2 * NS], comb[:, fd:2 * fd],
                     start=False, stop=False)
            T.matmul(o14[:, :], w4[:, 2 * NS:3 * NS], comb[:, 2 * fd:3 * fd],
                     start=False, stop=False)
            T.matmul(o14[:, :], w4[:, 3 * NS:4 * NS], comb[:, 3 * fd:4 * fd],
                     start=False, stop=False)
            T.matmul(o14[:, :], w4[:, 4 * NS:5 * NS], st["pp"][:, :],
                     start=False, stop=True)

            o14sb = sb.tile([NS, fd], F32, tag="o14sb", name=f"o14sb_{t}")
            S.activation(o14sb[:, :], o14[:, :], AF.Identity, bias=b4s[:, 0:1])

            # ---- transpose to points-on-partitions ----
            qt = pa.tile([128, NS * nchunks], F32, tag="pa", name=f"qt_{t}")
            for ci in range(nchunks):
                w = min(128, fd - ci * 128)
                T.transpose(qt[0:w, ci * NS:(ci + 1) * NS],
                            o14sb[:, ci * 128:ci * 128 + w],
                            ident[:, :])
            gw = min(128, fd - (nchunks - 1) * 128)
            qoff = t * NS * 4
            if gw == 128:
                S.copy(qall[:, qoff:qoff + NS * nchunks], qt[:, :])
            else:
                if nchunks > 1:
                    S.copy(qall[:, qoff:qoff + NS * (nchunks - 1)],
                           qt[:, 0:NS * (nchunks - 1)])
                S.copy(
                    qall[0:gw, qoff + NS * (nchunks - 1):qoff + NS * nchunks],
                    qt[0:gw, NS * (nchunks - 1):NS * nchunks])

        sts = {}
        prev_ts = []
        for p in range(0, DBG_NT, 2):
            ts = [t for t in (p, p + 1) if t < DBG_NT]
            for t in ts:
                sts[t] = emit_el1(t)
            for t in ts:
                emit_hidden(t, 0, sts[t])
            for t in prev_ts:
                emit_tail(t, sts[t])
            for t in ts:
                emit_hidden(t, 1, sts[t])
            for t in ts:
                emit_hidden(t, 2, sts[t])
            prev_ts = ts
        for t in prev_ts:
            emit_tail(t, sts[t])

        # ---- pointwise loss phase on [128, NCH] views ----
        def _pointwise():
            def qv(j):
                return qall[:, j:NS * NCH:NS]

            _ctr = [0]

            def new():
                _ctr[0] += 1
                return scr.tile([128, NCH], F32, tag="scr", name=f"scr{_ctr[0]}")

            def tt(a, b, op, eng=V):
                o = new()
                eng.tensor_tensor(o[:, :], a, b, op)
                return o

            A = tt(qv(4), qv(8), ALU.add, G)             # ux+uy
            B = tt(qv(5), qv(9), ALU.add, V)             # vx+vy
            uv1 = tt(qv(1), A[:, :], ALU.mult, G)        # v*(ux+uy)
            uv2 = tt(qv(0), B[:, :], ALU.mult, V)        # u*(vx+vy)
            uvxy = tt(uv1[:, :], uv2[:, :], ALU.add, V)

            def stt_nu(zz, eng):
                o = new()
                eng.scalar_tensor_tensor(o[:, :], qv(3), NU, zz, ALU.add,
                                         ALU.mult)
                return o

            t1 = stt_nu(qv(12), V)               # (nut+NU)*Lu
            t3 = tt(qv(7), qv(4), ALU.mult, V)   # nux*ux
            t4 = tt(qv(11), qv(8), ALU.mult, G)  # nuy*uy
            a1 = tt(uvxy[:, :], qv(6), ALU.add, V)
            a3 = tt(t3[:, :], t4[:, :], ALU.add, G)
            a4 = tt(t1[:, :], a3[:, :], ALU.add, V)
            f_u = tt(a1[:, :], a4[:, :], ALU.subtract, V)

            t5 = stt_nu(qv(13), V)               # (nut+NU)*Lv
            t7 = tt(qv(7), qv(5), ALU.mult, V)   # nux*vx
            t8 = tt(qv(11), qv(9), ALU.mult, G)  # nuy*vy
            b1 = tt(uvxy[:, :], qv(10), ALU.add, V)
            b3 = tt(t7[:, :], t8[:, :], ALU.add, G)
            b4 = tt(t5[:, :], b3[:, :], ALU.add, V)
            f_v = tt(b1[:, :], b4[:, :], ALU.subtract, V)

            ic = tt(qv(4), qv(9), ALU.add, G)

            t1b = new()
            V.tensor_scalar(t1b[:, :], qv(0), cnb[:, 0:1], cnb[:, 1:2],
                            ALU.mult, ALU.add)
            xnb = new()
            V.tensor_scalar(xnb[:, :], tins[:, 0:NCH], cnb[:, 2:3], cnb[:, 3:4],
                            ALU.mult, ALU.add)
            t2b = new()
            V.tensor_scalar(t2b[:, :], qv(1), cnb[:, 4:5], cnb[:, 5:6],
                            ALU.mult, ALU.add)
            ynb = new()
            V.tensor_scalar(ynb[:, :], tins[:, NCH:2 * NCH], cnb[:, 6:7],
                            cnb[:, 7:8], ALU.mult, ALU.add)
            m1 = tt(t1b[:, :], xnb[:, :], ALU.mult, V)
            m2 = tt(t2b[:, :], ynb[:, :], ALU.mult, G)
            bc0 = tt(m1[:, :], m2[:, :], ALU.add, V)
            bc = tt(bc0[:, :], mask[:, :], ALU.mult, V)

            du = tt(tins[:, 2 * NCH:3 * NCH], qv(0), ALU.subtract, V)
            dv = tt(tins[:, 3 * NCH:4 * NCH], qv(1), ALU.subtract, G)
            dp = tt(tins[:, 4 * NCH:5 * NCH], qv(2), ALU.subtract, V)
            dnut = tt(tins[:, 5 * NCH:6 * NCH], qv(3), ALU.subtract, G)

            for k, val in enumerate([f_u, f_v, bc, ic, du, dv, dp, dnut]):
                o = new()
                S.activation(o[:, :], val[:, :], AF.Square,
                             accum_out=souts[:, k:k + 1])

            nc.sync.dma_start(out=d_out[:, :], in_=souts[:, :])

        if DBG_PW:
            _pointwise()
        else:
            nc.sync.dma_start(out=d_out[:, :], in_=qall[:, 0:8])

    nc.compile()
    return nc


def _prep_core(inputs, c):
    s = slice(c * NPC, (c + 1) * NPC)
    f32 = np.float32
    col = lambda k: np.asarray(inputs[k], f32)[s, 0]
    feat = np.ascontiguousarray(np.stack([
        col("x"), col("y"), col("x_normal"), col("y_normal"), col("sdf"),
        col("gamma_1"), col("gamma_2"), col("gamma_3")]))
    W = [np.asarray(inputs[f"W{i}"], f32) for i in range(5)]
    b = [np.asarray(inputs[f"b{i}"], f32) for i in range(5)]
    cn = np.asarray(inputs["coef_norm"], f32)

    w0x, w0y = W[0][0, :], W[0][1, :]
    w0s = 2.0 * (w0x * w0x + w0y * w0y)
    wh = np.concatenate([W[1], W[2], W[3]], axis=1)
    wx = np.concatenate([W[1] * w0x[:, None], W[1] * w0y[:, None],
                         W[1] * w0s[:, None]], axis=1)
    w4sp = np.zeros((128, 5 * NS), f32)
    w4sp[:, 0:4] = W[4]                       # slot th -> rows 0:4
    w4sp[:, NS + 4:NS + 8] = W[4]             # slot t_x -> rows 4:8
    w4sp[:, 2 * NS + 8:2 * NS + 12] = W[4]    # slot t_y -> rows 8:12
    w4sp[:, 3 * NS + 12:3 * NS + 14] = -W[4][:, 0:2]  # slot s -> rows 12:14
    w4sp[:, 4 * NS + 12:4 * NS + 14] = W[4][:, 0:2]   # slot pp -> rows 12:14

    bias = np.stack([b[0], b[1], b[2], b[3]], axis=1)
    b4s = np.concatenate([b[4], np.zeros(NS - 4, f32)])[:, None]
    cnv = np.array([cn[3, 0] + EPS, cn[2, 0], cn[1, 5] + EPS, cn[0, 5],
                    cn[3, 1] + EPS, cn[2, 1], cn[1, 6] + EPS, cn[0, 6]], f32)
    cnb = np.broadcast_to(cnv, (128, 8)).copy()

    def tcol(k):
        a = np.zeros(NCH * 128, f32)
        a[:NPC] = col(k)
        return a.reshape(NCH, 128).T  # [128, NCH]

    tins = np.ascontiguousarray(np.concatenate(
        [tcol("x_normal"), tcol("y_normal"), tcol("u0"), tcol("v0"),
         tcol("p0"), tcol("nut0")], axis=1))
    m = np.zeros(NCH * 128, f32)
    m[:NPC] = 1.0
    mask = np.ascontiguousarray(m.reshape(NCH, 128).T)

    bf = lambda a: np.ascontiguousarray(a.astype(BF_NP))
    return {
        "featb": bf(feat), "w0b": bf(W[0]), "wh": bf(wh), "wx": bf(wx),
        "w2n": bf(-W[2]),
        "w4sp": bf(w4sp),
        "bias": np.ascontiguousarray(bias), "b4s": b4s,
        "cnb": cnb, "tins": tins, "mask": mask,
        "ident": np.eye(NS, dtype=f32),
    }


def _get_nc():
    if "nc" not in _CACHE:
        _CACHE["nc"] = _build()
    return _CACHE["nc"]


def run_device(inputs, **kw):
    nc = _get_nc()
    in_maps = [_prep_core(inputs, c) for c in range(NCORES)]
    res = run_bass_kernel_spmd(nc, in_maps, core_ids=list(range(NCORES)), **kw)
    return res


def _combine(results):
    S = np.stack([r["sout"] for r in results]).astype(np.float64)  # [8,128,8]
    m = S.sum(axis=(0, 1)) / NPTS
    rans, bcl, icl = m[0] + m[1], m[2], m[3]
    ul, vl, pl, nl = m[4], m[5], m[6], m[7]
    inlet = ul + vl + pl + nl
    total = rans + bcl + inlet + icl
    return np.array([total, rans, bcl, inlet, icl, ul, vl, pl, nl],
                    dtype=np.float32)


def kernel(**inputs):
    res = run_device(inputs)
    return _combine(res.results)
